# revision 17
# baseline (speedup 1.0000x reference)
"""Trainium2 Bass kernel for nn_DataEmbedding_Stats (v2).

Computation: rolling-window stats (window=24, replicate-padded) over
x (B,S,7) -> 35 features -> circular conv1d(k=3) -> (B,S,512).

Strategy (8 NeuronCores, data parallel over batch, 4 batches/core):
 - 2 super-slabs (seq 0..2079 / 2080..4095), X loaded as bf16 via
   SWDGE cast-DMA into [112, E] tiles, partition = 28j + 7b + c.
 - rolling stats via log-doubling shifted tensor_tensor ladders,
   all in bf16 (2x DVE rate); sum/sq/max/min chains + var/std.
 - hop1: STK [112, 5*E] -> ST2A [28, 5*4104] (col = seq+2), host
   wrap stats for seq 4094/4095, high-wrap for seq 0/1.
 - hop2: F3 [106, 2048] per (batch, group): 3 tap-gathers from ST2A
   (4KB runs) + ones row (bias); group g covers pos 2048g..2048g+2047.
 - matmul weight-stationary: lhsT = wt[:,128dc:+128] (bias row 105),
   rhs = F3 slice [106, 512]; PSUM [128,1024] f32 pairs; drains
   (f32->bf16 cast) split 5:3 across ACT/DVE; output DMA per
   (b,g,dc) [128,2048] bf16 into flat y [128, 65536] (4KB runs);
   host re-assembles (b, s, d) and upcasts.
"""

import numpy as np

try:
    import concourse.bass as bass  # noqa: F401
except ImportError:
    import sys

    for _p in ("/opt/trn_rl_repo", "/root/.axon_site/_ro/trn_rl_repo"):
        if _p not in sys.path:
            sys.path.insert(0, _p)

B, S, C, W, D = 32, 4096, 7, 24, 512
NCORES = 8
BSH = B // NCORES          # batches per core
NF = 5 * C                 # 35 features
K = 3 * NF + 1             # 106 contraction rows (ones/bias row last)
HALO = W - 1               # 23
STW = 4104                 # per-stat block width in ST2A (col = seq + 2)
F3W = 2048                 # F3 cols = positions per group
NG = 2                     # output groups of 2048 positions
NDC = 4                    # d_model chunks of 128
# super-slabs: (seq base, chunk len, n chunks); E = chunk + HALO
SSPEC = ((0, 520, 4), (2080, 504, 4))

_CACHE = {}


def _build():
    import concourse.bacc as bacc
    import concourse.tile as tile
    from concourse import mybir

    f32 = mybir.dt.float32
    bf16 = mybir.dt.bfloat16
    Alu = mybir.AluOpType
    Act = mybir.ActivationFunctionType

    nc = bacc.Bacc(
        "TRN2",
        target_bir_lowering=False,
        debug=False,
        enable_asserts=False,
        num_devices=NCORES,
    )

    x_d = nc.dram_tensor("x", (BSH, C, S), f32, kind="ExternalInput")
    wt_d = nc.dram_tensor("wt", (K, D), bf16, kind="ExternalInput")
    ones_d = nc.dram_tensor("ones", (1, F3W), bf16, kind="ExternalInput")
    wrap_d = nc.dram_tensor("wrap", (28, 10), bf16, kind="ExternalInput")
    y_d = nc.dram_tensor(
        "y", (128, BSH * NG * NDC * F3W), bf16, kind="ExternalOutput"
    )

    with tile.TileContext(nc) as tc:
        with (
            tc.tile_pool(name="const", bufs=1) as pco,
            tc.tile_pool(name="xp", bufs=1) as pxx,
            tc.tile_pool(name="scr", bufs=1) as pscr,
            tc.tile_pool(name="lad", bufs=1) as plad,
            tc.tile_pool(name="stk", bufs=1) as pstk,
            tc.tile_pool(name="st2", bufs=1) as pst2,
            tc.tile_pool(name="f3p", bufs=8) as pf3,
            tc.tile_pool(name="psum", bufs=4, space="PSUM") as pps,
            tc.tile_pool(name="outp", bufs=4) as pout,
        ):
            wt = pco.tile([K, D], bf16, tag="wt")
            ST2A = pst2.tile([32, 5 * STW], bf16, tag="ST2A")
            XF = [None] * 2
            XB = [None] * 2
            STK = [None] * 2
            F3 = [[None] * NG for _ in range(BSH)]
            STAGE = [[None] * NG for _ in range(BSH)]

            def load_x(ss):
                # f32 loads on sync (HWDGE: fast issue, no SWDGE warmup)
                base, ch, nj = SSPEC[ss]
                E = ch + HALO
                Xf = pxx.tile([112, E], f32, tag=f"Xf{ss}", name=f"Xf{ss}")
                XF[ss] = Xf
                for j in range(nj):
                    s0 = base + ch * j
                    if ss == 0 and j == 0:
                        nc.sync.dma_start(
                            Xf[0:28, HALO:E],
                            x_d.ap()[:, :, 0:ch].rearrange("b c q -> (b c) q"),
                        )
                        # init halo cols with dummy bytes (replaced by the
                        # replicate op on XB after the cast)
                        nc.sync.dma_start(
                            Xf[0:28, 0:HALO],
                            x_d.ap()[:, :, 0:HALO].rearrange(
                                "b c q -> (b c) q"
                            ),
                        )
                    else:
                        nc.sync.dma_start(
                            Xf[28 * j : 28 * j + 28, :],
                            x_d.ap()[
                                :, :, s0 - HALO : s0 + ch
                            ].rearrange("b c q -> (b c) q"),
                        )

            def cast_x(ss):
                base, ch, nj = SSPEC[ss]
                E = ch + HALO
                Xn = pxx.tile([112, E], bf16, tag=f"X{ss}", name=f"X{ss}")
                XB[ss] = Xn
                nc.vector.tensor_copy(Xn[0:112, 0:E], XF[ss][0:112, 0:E])

            def halo0():
                Xn = XB[0]
                # replicate x[b,0,c] into halo cols 0..22 (scalar2 AP must
                # be f32, so stage the column through a tiny f32 tile)
                hc = pscr.tile([28, 1], f32, tag="haloc")
                nc.vector.tensor_copy(hc[0:28, 0:1], Xn[0:28, HALO : HALO + 1])
                nc.vector.tensor_scalar(
                    Xn[0:28, 0:HALO],
                    Xn[0:28, HALO : 2 * HALO],
                    0.0,
                    hc[0:28, 0:1],
                    Alu.mult,
                    Alu.add,
                )

            shifts = (1, 3, 7, 15)
            deltas = (1, 2, 4, 8)

            def mk_lad(ss, chains):
                _, ch, _ = SSPEC[ss]
                E = ch + HALO
                return {
                    c: [
                        plad.tile([112, E], bf16, tag=f"{c}{l}_{ss}",
                                  name=f"{c}{l}_{ss}")
                        for l in range(4)
                    ]
                    for c in chains
                }

            def tt(dst, d0, a, a0, bs, b0, op, E):
                nc.vector.tensor_tensor(
                    dst[0:112, d0:E], a[0:112, a0 : a0 + E - d0],
                    bs[0:112, b0 : b0 + E - d0], op,
                )

            def chain_levels(lad, src0, op, ss):
                _, ch, _ = SSPEC[ss]
                E = ch + HALO
                for l in range(4):
                    s, dl = shifts[l], deltas[l]
                    src = src0 if l == 0 else lad[l - 1]
                    tt(lad[l], s, src, s, src, s - dl, op, E)

            def stats_sumsq(ss):
                """sum + sq chains and their finals (DVE) + SQX (ACT)."""
                _, ch, _ = SSPEC[ss]
                E = ch + HALO
                Xn = XB[ss]
                SQX = pscr.tile([112, E], bf16, tag=f"SQX{ss}", name=f"SQX{ss}")
                nc.scalar.square(SQX[0:112, 0:E], Xn[0:112, 0:E])
                lad = mk_lad(ss, "CQ")
                C_, Q_ = lad["C"], lad["Q"]
                # interleave the two chains for DVE pipeline independence
                for l in range(4):
                    s, dl = shifts[l], deltas[l]
                    csrc = Xn if l == 0 else C_[l - 1]
                    qsrc = SQX if l == 0 else Q_[l - 1]
                    tt(C_[l], s, csrc, s, csrc, s - dl, Alu.add, E)
                    tt(Q_[l], s, qsrc, s, qsrc, s - dl, Alu.add, E)
                S24 = pscr.tile([112, E], bf16, tag=f"S24_{ss}", name=f"S24_{ss}")
                SQ24 = pscr.tile([112, E], bf16, tag=f"SQ24_{ss}",
                                 name=f"SQ24_{ss}")
                tt(S24, HALO, C_[3], HALO, C_[2], 7, Alu.add, E)
                tt(SQ24, HALO, Q_[3], HALO, Q_[2], 7, Alu.add, E)
                return S24, SQ24

            def stats_rest(ss, S24, SQ24):
                """max/min chains, var/std path, stat blocks -> STK."""
                _, ch, _ = SSPEC[ss]
                E = ch + HALO
                Xn = XB[ss]
                stk = pstk.tile([112, 5 * E], bf16, tag=f"STK{ss}",
                                name=f"STK{ss}")
                STK[ss] = stk

                def blk(t):
                    return stk[0:112, E * t + HALO : E * (t + 1)]

                lad = mk_lad(ss, "MN")
                M_, N_ = lad["M"], lad["N"]
                for l in range(4):
                    s, dl = shifts[l], deltas[l]
                    msrc = Xn if l == 0 else M_[l - 1]
                    nsrc = Xn if l == 0 else N_[l - 1]
                    tt(M_[l], s, msrc, s, msrc, s - dl, Alu.max, E)
                    tt(N_[l], s, nsrc, s, nsrc, s - dl, Alu.min, E)
                nc.vector.tensor_tensor(
                    blk(2), M_[3][0:112, HALO:E], M_[2][0:112, 7 : E - 16],
                    Alu.max,
                )
                nc.vector.tensor_tensor(
                    blk(3), N_[3][0:112, HALO:E], N_[2][0:112, 7 : E - 16],
                    Alu.min,
                )
                # x and mean (raw S24; 1/24 folded into weights) blocks
                nc.vector.tensor_copy(blk(0), Xn[0:112, HALO:E])
                nc.vector.tensor_copy(blk(1), S24[0:112, HALO:E])
                # std = sqrt(max(SQ24 - S24^2/24, 0)/23)
                T4 = pscr.tile([112, E], bf16, tag=f"T4_{ss}", name=f"T4_{ss}")
                VV = pscr.tile([112, E], bf16, tag=f"VV_{ss}", name=f"VV_{ss}")
                VC = pscr.tile([112, E], bf16, tag=f"VC_{ss}", name=f"VC_{ss}")
                nc.scalar.activation(
                    T4[0:112, HALO:E], S24[0:112, HALO:E], Act.Square, 0.0,
                    float(W**-0.5),
                )
                tt(VV, HALO, SQ24, HALO, T4, HALO, Alu.subtract, E)
                nc.vector.tensor_scalar(
                    VC[0:112, HALO:E], VV[0:112, HALO:E], 0.0, None, Alu.max
                )
                nc.scalar.activation(
                    blk(4), VC[0:112, HALO:E], Act.Sqrt, 0.0, 1.0 / (W - 1)
                )

            def hop1(ss):
                base, ch, nj = SSPEC[ss]
                E = ch + HALO
                stk = STK[ss]
                for j in range(nj):
                    src = stk[28 * j : 28 * j + 28, :].rearrange(
                        "g (t m) -> g t m", m=E
                    )[:, :, HALO:E]
                    c0 = 2 + base + ch * j
                    dst = ST2A[0:28, :].rearrange("g (t m) -> g t m", m=STW)[
                        :, :, c0 : c0 + ch
                    ]
                    nc.sync.dma_start(dst, src)

            def high_wrap():
                # seq 0,1 -> ST2A cols 4098..4099 (circular high wrap)
                nc.sync.dma_start(
                    ST2A[0:28, :].rearrange("g (t m) -> g t m", m=STW)[
                        :, :, S + 2 : S + 4
                    ],
                    STK[0][0:28, :].rearrange(
                        "g (t m) -> g t m", m=SSPEC[0][1] + HALO
                    )[:, :, HALO : HALO + 2],
                )

            def wrap_low():
                # host stats for seq 4094/4095 -> ST2A cols 0..1
                nc.gpsimd.dma_start(
                    ST2A[0:28, :].rearrange("g (t m) -> g t m", m=STW)[:, :, 0:2],
                    wrap_d.ap().rearrange("g (t m) -> g t m", m=2),
                )

            def build_f3(b, g):
                f3 = F3[b][g]
                for k in range(3):
                    # F3 row 35k + 5c + t, col q <- ST2A col 2048g + k + 1 + q
                    c0 = F3W * g + k + 1
                    src = ST2A[7 * b : 7 * b + 7, :].rearrange(
                        "c (t m) -> c t m", m=STW
                    )[:, :, c0 : c0 + F3W]
                    nc.sync.dma_start(f3[35 * k : 35 * k + 35, :], src)

            # drain engine rotation: 5 ACT : 3 DVE per 8
            DVE_SLOTS = (1, 4, 7)

            def mm_group(g):
                di = 0
                for b in range(BSH):
                    f3 = F3[b][g]
                    stage = pout.tile(
                        [128, NDC * F3W], bf16, tag="stage",
                        name=f"stage_{b}_{g}",
                    )
                    STAGE[b][g] = stage
                    for dc in range(NDC):
                        for ph in range(2):
                            ps = pps.tile([128, 1024], f32, tag="ps")
                            for h in range(2):
                                q0 = ph * 1024 + h * 512
                                nc.tensor.matmul(
                                    ps[:, 512 * h : 512 * (h + 1)],
                                    wt[0:K, 128 * dc : 128 * (dc + 1)],
                                    f3[0:K, q0 : q0 + 512],
                                    start=True,
                                    stop=True,
                                )
                            col = dc * F3W + ph * 1024
                            if di % 8 in DVE_SLOTS:
                                nc.vector.tensor_copy(
                                    stage[:, col : col + 1024], ps[:, 0:1024]
                                )
                            else:
                                nc.scalar.copy(
                                    stage[:, col : col + 1024], ps[:, 0:1024]
                                )
                            di += 1
                        colbase = ((b * NG + g) * NDC + dc) * F3W
                        nc.gpsimd.dma_start(
                            y_d.ap()[:, colbase : colbase + F3W],
                            stage[:, dc * F3W : (dc + 1) * F3W],
                        )

            # ---------------- pipeline
            # wt first on gpsimd: absorbs the one-time SWDGE ucode warmup
            nc.gpsimd.dma_start(wt[:], wt_d.ap())
            wrap_low()
            # pre-create F3 tiles; ones (bias) rows loaded early on sync
            for b in range(BSH):
                for g in range(NG):
                    F3[b][g] = pf3.tile(
                        [K, F3W], bf16, tag="F3", name=f"f3_{b}_{g}"
                    )
                    nc.sync.dma_start(F3[b][g][K - 1 : K, :], ones_d.ap())
            load_x(0)
            load_x(1)
            cast_x(0)
            halo0()
            cast_x(1)
            # SS0 full stats
            S24_0, SQ24_0 = stats_sumsq(0)
            stats_rest(0, S24_0, SQ24_0)
            hop1(0)
            high_wrap()
            for b in range(BSH):
                build_f3(b, 0)
            # SS1 sum/sq chains fill the DVE gap before G0 drains are ready
            S24_1, SQ24_1 = stats_sumsq(1)
            mm_group(0)
            stats_rest(1, S24_1, SQ24_1)
            hop1(1)
            for b in range(BSH):
                build_f3(b, 1)
            mm_group(1)

    nc.compile()
    return nc


def _prep_host(W_conv, b_conv):
    import ml_dtypes

    wt = np.empty((K, D), np.float32)
    wkf = np.ascontiguousarray(W_conv.transpose(2, 1, 0)).copy()  # (3, 35, 512)
    wkf[:, C : 2 * C, :] *= 1.0 / W  # fold mean = S24/24 into weights
    # row order within a tap: 5c + t (channel-major, matches F3 gather)
    wkf = wkf.reshape(3, 5, C, D).transpose(0, 2, 1, 3).reshape(3, NF, D)
    wt[: K - 1] = wkf.reshape(3 * NF, D)
    wt[K - 1] = b_conv.astype(np.float32)
    return wt.astype(ml_dtypes.bfloat16)


def _run(x, W_conv, b_conv, trace=False, **kw):
    from concourse import bass_utils

    if "nc" not in _CACHE:
        _CACHE["nc"] = _build()
    nc = _CACHE["nc"]

    wt = _prep_host(np.asarray(W_conv), np.asarray(b_conv))
    import ml_dtypes

    ones = np.ones((1, F3W), ml_dtypes.bfloat16)
    x = np.asarray(x, np.float32)
    # host stats for the circular-wrap cols (seq 4094/4095), [28, 10] per
    # core: row 7b+c, col 2t+e (t: x,sum,max,min,std; e: seq 4094+e)
    wraps = []
    for i in range(NCORES):
        wr = np.empty((BSH, C, 5, 2), np.float32)
        for b in range(BSH):
            for e in range(2):
                win = x[BSH * i + b, S - W - 1 + e : S - 1 + e, :]  # (24, 7)
                s24 = win.sum(0)
                var = np.maximum(
                    (win * win).sum(0) - s24 * s24 / W, 0.0
                ) / (W - 1)
                wr[b, :, 0, e] = x[BSH * i + b, S - 2 + e, :]
                wr[b, :, 1, e] = s24
                wr[b, :, 2, e] = win.max(0)
                wr[b, :, 3, e] = win.min(0)
                wr[b, :, 4, e] = np.sqrt(var)
        wraps.append(
            wr.reshape(BSH * C, 10).astype(ml_dtypes.bfloat16)
        )
    xt = np.ascontiguousarray(x.transpose(0, 2, 1))  # (B, C, S)
    in_maps = [
        {
            "x": xt[BSH * i : BSH * (i + 1)],
            "wt": wt,
            "ones": ones,
            "wrap": wraps[i],
        }
        for i in range(NCORES)
    ]
    res = bass_utils.run_bass_kernel_spmd(
        nc, in_maps, core_ids=list(range(NCORES)), trace=trace, **kw
    )
    outs = []
    for r in res.results:
        arr = np.asarray(r["y"], np.float32)  # (128, 65536)
        arr = arr.reshape(128, BSH, NG, NDC, F3W)
        # out[b, g*2048 + s, dc*128 + p]
        outs.append(
            np.ascontiguousarray(arr.transpose(1, 2, 4, 3, 0)).reshape(
                BSH, S, D
            )
        )
    out = np.concatenate(outs, axis=0)
    return out, res


def kernel(x, x_mark=None, W_conv=None, b_conv=None, **_unused):
    out, _ = _run(x, W_conv, b_conv, trace=False)
    return out


# revision 24
# speedup vs baseline: 1.0208x; 1.0208x over previous
"""Trainium2 Bass kernel for nn_DataEmbedding_Stats (v2).

Computation: rolling-window stats (window=24, replicate-padded) over
x (B,S,7) -> 35 features -> circular conv1d(k=3) -> (B,S,512).

Strategy (8 NeuronCores, data parallel over batch, 4 batches/core):
 - 2 super-slabs (seq 0..2079 / 2080..4095), X loaded as bf16 via
   SWDGE cast-DMA into [112, E] tiles, partition = 28j + 7b + c.
 - rolling stats via log-doubling shifted tensor_tensor ladders,
   all in bf16 (2x DVE rate); sum/sq/max/min chains + var/std.
 - hop1: STK [112, 5*E] -> ST2A [28, 5*4104] (col = seq+2), host
   wrap stats for seq 4094/4095, high-wrap for seq 0/1.
 - hop2: F3 [106, 2048] per (batch, group): 3 tap-gathers from ST2A
   (4KB runs) + ones row (bias); group g covers pos 2048g..2048g+2047.
 - matmul weight-stationary: lhsT = wt[:,128dc:+128] (bias row 105),
   rhs = F3 slice [106, 512]; PSUM [128,1024] f32 pairs; drains
   (f32->bf16 cast) split 5:3 across ACT/DVE; output DMA per
   (b,g,dc) [128,2048] bf16 into flat y [128, 65536] (4KB runs);
   host re-assembles (b, s, d) and upcasts.
"""

import numpy as np

try:
    import concourse.bass as bass  # noqa: F401
except ImportError:
    import sys

    for _p in ("/opt/trn_rl_repo", "/root/.axon_site/_ro/trn_rl_repo"):
        if _p not in sys.path:
            sys.path.insert(0, _p)

B, S, C, W, D = 32, 4096, 7, 24, 512
NCORES = 8
BSH = B // NCORES          # batches per core
NF = 5 * C                 # 35 features
K = 3 * NF + 1             # 106 contraction rows (ones/bias row last)
HALO = W - 1               # 23
STW = 4104                 # per-stat block width in ST2A (col = seq + 2)
F3W = 2048                 # F3 cols = positions per group
NG = 2                     # output groups of 2048 positions
NDC = 4                    # d_model chunks of 128
# super-slabs: (seq base, chunk len, n chunks); E = chunk + HALO
SSPEC = ((0, 520, 4), (2080, 504, 4))

_CACHE = {}


def _build():
    import concourse.bacc as bacc
    import concourse.tile as tile
    from concourse import mybir

    f32 = mybir.dt.float32
    bf16 = mybir.dt.bfloat16
    Alu = mybir.AluOpType
    Act = mybir.ActivationFunctionType

    nc = bacc.Bacc(
        "TRN2",
        target_bir_lowering=False,
        debug=False,
        enable_asserts=False,
        num_devices=NCORES,
    )

    x_d = nc.dram_tensor("x", (BSH, C, S), f32, kind="ExternalInput")
    wt_d = nc.dram_tensor("wt", (K, D), bf16, kind="ExternalInput")
    ones_d = nc.dram_tensor("ones", (1, F3W), bf16, kind="ExternalInput")
    wrap_d = nc.dram_tensor("wrap", (28, 10), bf16, kind="ExternalInput")
    y_d = nc.dram_tensor(
        "y", (128, BSH * NG * NDC * F3W), bf16, kind="ExternalOutput"
    )

    with tile.TileContext(nc) as tc:
        with (
            tc.tile_pool(name="const", bufs=1) as pco,
            tc.tile_pool(name="xp", bufs=1) as pxx,
            tc.tile_pool(name="scr", bufs=1) as pscr,
            tc.tile_pool(name="lad", bufs=1) as plad,
            tc.tile_pool(name="stk", bufs=1) as pstk,
            tc.tile_pool(name="st2", bufs=1) as pst2,
            tc.tile_pool(name="f3p", bufs=8) as pf3,
            tc.tile_pool(name="psum", bufs=2, space="PSUM") as pps,
            tc.tile_pool(name="outp", bufs=4) as pout,
        ):
            wt = pco.tile([K, D], bf16, tag="wt")
            ST2A = pst2.tile([32, 5 * STW], bf16, tag="ST2A")
            XB = [None] * 2
            STK = [None] * 2
            F3 = [[None] * NG for _ in range(BSH)]
            STAGE = [[None] * NG for _ in range(BSH)]

            def load_x(ss):
                # SWDGE (gpsimd) cast-loads: f32 HBM -> bf16 SBUF
                base, ch, nj = SSPEC[ss]
                E = ch + HALO
                Xn = pxx.tile([112, E], bf16, tag=f"X{ss}", name=f"X{ss}")
                XB[ss] = Xn
                for j in range(nj):
                    s0 = base + ch * j
                    if ss == 0 and j == 0:
                        nc.gpsimd.dma_start(
                            Xn[0:28, HALO:E],
                            x_d.ap()[:, :, 0:ch].rearrange("b c q -> (b c) q"),
                        )
                    else:
                        nc.gpsimd.dma_start(
                            Xn[28 * j : 28 * j + 28, :],
                            x_d.ap()[
                                :, :, s0 - HALO : s0 + ch
                            ].rearrange("b c q -> (b c) q"),
                        )

            def halo0():
                Xn = XB[0]
                # replicate x[b,0,c] into halo cols 0..22 (scalar2 AP must
                # be f32, so stage the column through a tiny f32 tile)
                hc = pscr.tile([28, 1], f32, tag="haloc")
                nc.vector.tensor_copy(hc[0:28, 0:1], Xn[0:28, HALO : HALO + 1])
                nc.vector.tensor_scalar(
                    Xn[0:28, 0:HALO],
                    Xn[0:28, HALO : 2 * HALO],
                    0.0,
                    hc[0:28, 0:1],
                    Alu.mult,
                    Alu.add,
                )

            shifts = (1, 3, 7, 15)
            deltas = (1, 2, 4, 8)

            def mk_lad(ss, chains):
                _, ch, _ = SSPEC[ss]
                E = ch + HALO
                return {
                    c: [
                        plad.tile([112, E], bf16, tag=f"{c}{l}_{ss}",
                                  name=f"{c}{l}_{ss}")
                        for l in range(4)
                    ]
                    for c in chains
                }

            def tt(dst, d0, a, a0, bs, b0, op, E):
                nc.vector.tensor_tensor(
                    dst[0:112, d0:E], a[0:112, a0 : a0 + E - d0],
                    bs[0:112, b0 : b0 + E - d0], op,
                )

            def chain_levels(lad, src0, op, ss):
                _, ch, _ = SSPEC[ss]
                E = ch + HALO
                for l in range(4):
                    s, dl = shifts[l], deltas[l]
                    src = src0 if l == 0 else lad[l - 1]
                    tt(lad[l], s, src, s, src, s - dl, op, E)

            def stats_sumsq(ss):
                """sum + sq via f32-state prefix scans + shifted subtracts."""
                _, ch, _ = SSPEC[ss]
                E = ch + HALO
                Xn = XB[ss]
                SQX = pscr.tile([112, E], bf16, tag=f"SQX{ss}", name=f"SQX{ss}")
                nc.scalar.square(SQX[0:112, 0:E], Xn[0:112, 0:E])
                Pt = pscr.tile([112, E + 1], f32, tag=f"Pt{ss}", name=f"Pt{ss}")
                Qt = pscr.tile([112, E + 1], f32, tag=f"Qt{ss}", name=f"Qt{ss}")
                nc.vector.memset(Pt[0:112, 0:1], 0.0)
                nc.vector.memset(Qt[0:112, 0:1], 0.0)
                nc.vector.tensor_tensor_scan(
                    Pt[0:112, 1 : E + 1], Xn[0:112, 0:E], Xn[0:112, 0:E],
                    0.0, Alu.add, Alu.bypass,
                )
                nc.vector.tensor_tensor_scan(
                    Qt[0:112, 1 : E + 1], SQX[0:112, 0:E], SQX[0:112, 0:E],
                    0.0, Alu.add, Alu.bypass,
                )
                S24 = pscr.tile([112, E], bf16, tag=f"S24_{ss}", name=f"S24_{ss}")
                SQ24 = pscr.tile([112, E], bf16, tag=f"SQ24_{ss}",
                                 name=f"SQ24_{ss}")
                # S24[t] = P[t] - P[t-24]  (Pt is the prefix with a 0 col)
                nc.vector.tensor_tensor(
                    S24[0:112, HALO:E], Pt[0:112, HALO + 1 : E + 1],
                    Pt[0:112, 0 : E - HALO], Alu.subtract,
                )
                nc.vector.tensor_tensor(
                    SQ24[0:112, HALO:E], Qt[0:112, HALO + 1 : E + 1],
                    Qt[0:112, 0 : E - HALO], Alu.subtract,
                )
                return S24, SQ24

            def stats_rest(ss, S24, SQ24):
                """max/min chains, var/std path, stat blocks -> STK."""
                _, ch, _ = SSPEC[ss]
                E = ch + HALO
                Xn = XB[ss]
                stk = pstk.tile([112, 5 * E], bf16, tag=f"STK{ss}",
                                name=f"STK{ss}")
                STK[ss] = stk

                def blk(t):
                    return stk[0:112, E * t + HALO : E * (t + 1)]

                lad = mk_lad(ss, "MN")
                M_, N_ = lad["M"], lad["N"]
                for l in range(4):
                    s, dl = shifts[l], deltas[l]
                    msrc = Xn if l == 0 else M_[l - 1]
                    nsrc = Xn if l == 0 else N_[l - 1]
                    tt(M_[l], s, msrc, s, msrc, s - dl, Alu.max, E)
                    tt(N_[l], s, nsrc, s, nsrc, s - dl, Alu.min, E)
                nc.vector.tensor_tensor(
                    blk(2), M_[3][0:112, HALO:E], M_[2][0:112, 7 : E - 16],
                    Alu.max,
                )
                nc.vector.tensor_tensor(
                    blk(3), N_[3][0:112, HALO:E], N_[2][0:112, 7 : E - 16],
                    Alu.min,
                )
                # x and mean (raw S24; 1/24 folded into weights) blocks
                nc.vector.tensor_copy(blk(0), Xn[0:112, HALO:E])
                nc.vector.tensor_copy(blk(1), S24[0:112, HALO:E])
                # std = sqrt(max(SQ24 - S24^2/24, 0)/23)
                T4 = pscr.tile([112, E], bf16, tag=f"T4_{ss}", name=f"T4_{ss}")
                VV = pscr.tile([112, E], bf16, tag=f"VV_{ss}", name=f"VV_{ss}")
                VC = pscr.tile([112, E], bf16, tag=f"VC_{ss}", name=f"VC_{ss}")
                nc.scalar.activation(
                    T4[0:112, HALO:E], S24[0:112, HALO:E], Act.Square, 0.0,
                    float(W**-0.5),
                )
                tt(VV, HALO, SQ24, HALO, T4, HALO, Alu.subtract, E)
                nc.vector.tensor_scalar(
                    VC[0:112, HALO:E], VV[0:112, HALO:E], 0.0, None, Alu.max
                )
                nc.scalar.activation(
                    blk(4), VC[0:112, HALO:E], Act.Sqrt, 0.0, 1.0 / (W - 1)
                )

            def hop1(ss):
                base, ch, nj = SSPEC[ss]
                E = ch + HALO
                stk = STK[ss]
                for j in range(nj):
                    src = stk[28 * j : 28 * j + 28, :].rearrange(
                        "g (t m) -> g t m", m=E
                    )[:, :, HALO:E]
                    c0 = 2 + base + ch * j
                    dst = ST2A[0:28, :].rearrange("g (t m) -> g t m", m=STW)[
                        :, :, c0 : c0 + ch
                    ]
                    nc.sync.dma_start(dst, src)

            def high_wrap():
                # seq 0,1 -> ST2A cols 4098..4099 (circular high wrap)
                nc.sync.dma_start(
                    ST2A[0:28, :].rearrange("g (t m) -> g t m", m=STW)[
                        :, :, S + 2 : S + 4
                    ],
                    STK[0][0:28, :].rearrange(
                        "g (t m) -> g t m", m=SSPEC[0][1] + HALO
                    )[:, :, HALO : HALO + 2],
                )

            def wrap_low():
                # host stats for seq 4094/4095 -> ST2A cols 0..1
                nc.gpsimd.dma_start(
                    ST2A[0:28, :].rearrange("g (t m) -> g t m", m=STW)[:, :, 0:2],
                    wrap_d.ap().rearrange("g (t m) -> g t m", m=2),
                )

            def build_f3(b, g):
                f3 = F3[b][g]
                for k in range(3):
                    # F3 row 35k + 5c + t, col q <- ST2A col 2048g + k + 1 + q
                    c0 = F3W * g + k + 1
                    src = ST2A[7 * b : 7 * b + 7, :].rearrange(
                        "c (t m) -> c t m", m=STW
                    )[:, :, c0 : c0 + F3W]
                    nc.sync.dma_start(f3[35 * k : 35 * k + 35, :], src)

            def mm_group(g):
                di = 0
                for b in range(BSH):
                    f3 = F3[b][g]
                    stage = pout.tile(
                        [128, NDC * F3W], bf16, tag="stage",
                        name=f"stage_{b}_{g}",
                    )
                    STAGE[b][g] = stage
                    for dc in range(NDC):
                        # 4-bank PSUM tile: 4 N=512 matmuls, one drain
                        ps = pps.tile([128, 2048], f32, tag="ps")
                        for ph in range(4):
                            nc.tensor.matmul(
                                ps[:, 512 * ph : 512 * (ph + 1)],
                                wt[0:K, 128 * dc : 128 * (dc + 1)],
                                f3[0:K, 512 * ph : 512 * ph + 512],
                                start=True,
                                stop=True,
                            )
                        col = dc * F3W
                        if di % 3 == 1:
                            nc.vector.tensor_copy(
                                stage[:, col : col + F3W], ps[:, 0:F3W]
                            )
                        else:
                            nc.scalar.copy(
                                stage[:, col : col + F3W], ps[:, 0:F3W]
                            )
                        di += 1
                        colbase = ((b * NG + g) * NDC + dc) * F3W
                        nc.gpsimd.dma_start(
                            y_d.ap()[:, colbase : colbase + F3W],
                            stage[:, dc * F3W : (dc + 1) * F3W],
                        )

            # ---------------- pipeline
            load_x(0)
            load_x(1)
            nc.gpsimd.dma_start(wt[:], wt_d.ap())
            wrap_low()
            # pre-create F3 tiles; ones (bias) rows loaded early on gpsimd
            for b in range(BSH):
                for g in range(NG):
                    F3[b][g] = pf3.tile(
                        [K, F3W], bf16, tag="F3", name=f"f3_{b}_{g}"
                    )
                    nc.gpsimd.dma_start(F3[b][g][K - 1 : K, :], ones_d.ap())
            halo0()
            # SS0 full stats
            S24_0, SQ24_0 = stats_sumsq(0)
            stats_rest(0, S24_0, SQ24_0)
            hop1(0)
            high_wrap()
            for b in range(BSH):
                build_f3(b, 0)
            # SS1 sum/sq chains fill the DVE gap before G0 drains are ready
            S24_1, SQ24_1 = stats_sumsq(1)
            mm_group(0)
            stats_rest(1, S24_1, SQ24_1)
            hop1(1)
            for b in range(BSH):
                build_f3(b, 1)
            mm_group(1)

    nc.compile()
    return nc


def _prep_host(W_conv, b_conv):
    import ml_dtypes

    wt = np.empty((K, D), np.float32)
    wkf = np.ascontiguousarray(W_conv.transpose(2, 1, 0)).copy()  # (3, 35, 512)
    wkf[:, C : 2 * C, :] *= 1.0 / W  # fold mean = S24/24 into weights
    # row order within a tap: 5c + t (channel-major, matches F3 gather)
    wkf = wkf.reshape(3, 5, C, D).transpose(0, 2, 1, 3).reshape(3, NF, D)
    wt[: K - 1] = wkf.reshape(3 * NF, D)
    wt[K - 1] = b_conv.astype(np.float32)
    return wt.astype(ml_dtypes.bfloat16)


def _run(x, W_conv, b_conv, trace=False, **kw):
    from concourse import bass_utils

    if "nc" not in _CACHE:
        _CACHE["nc"] = _build()
    nc = _CACHE["nc"]

    wt = _prep_host(np.asarray(W_conv), np.asarray(b_conv))
    import ml_dtypes

    ones = np.ones((1, F3W), ml_dtypes.bfloat16)
    x = np.asarray(x, np.float32)
    # host stats for the circular-wrap cols (seq 4094/4095), [28, 10] per
    # core: row 7b+c, col 2t+e (t: x,sum,max,min,std; e: seq 4094+e)
    wraps = []
    for i in range(NCORES):
        wr = np.empty((BSH, C, 5, 2), np.float32)
        for b in range(BSH):
            for e in range(2):
                win = x[BSH * i + b, S - W - 1 + e : S - 1 + e, :]  # (24, 7)
                s24 = win.sum(0)
                var = np.maximum(
                    (win * win).sum(0) - s24 * s24 / W, 0.0
                ) / (W - 1)
                wr[b, :, 0, e] = x[BSH * i + b, S - 2 + e, :]
                wr[b, :, 1, e] = s24
                wr[b, :, 2, e] = win.max(0)
                wr[b, :, 3, e] = win.min(0)
                wr[b, :, 4, e] = np.sqrt(var)
        wraps.append(
            wr.reshape(BSH * C, 10).astype(ml_dtypes.bfloat16)
        )
    xt = np.ascontiguousarray(x.transpose(0, 2, 1))  # (B, C, S)
    in_maps = [
        {
            "x": xt[BSH * i : BSH * (i + 1)],
            "wt": wt,
            "ones": ones,
            "wrap": wraps[i],
        }
        for i in range(NCORES)
    ]
    res = bass_utils.run_bass_kernel_spmd(
        nc, in_maps, core_ids=list(range(NCORES)), trace=trace, **kw
    )
    outs = []
    for r in res.results:
        arr = np.asarray(r["y"], np.float32)  # (128, 65536)
        arr = arr.reshape(128, BSH, NG, NDC, F3W)
        # out[b, g*2048 + s, dc*128 + p]
        outs.append(
            np.ascontiguousarray(arr.transpose(1, 2, 4, 3, 0)).reshape(
                BSH, S, D
            )
        )
    out = np.concatenate(outs, axis=0)
    return out, res


def kernel(x, x_mark=None, W_conv=None, b_conv=None, **_unused):
    out, _ = _run(x, W_conv, b_conv, trace=False)
    return out


# revision 25
# speedup vs baseline: 1.2610x; 1.2353x over previous
"""Trainium2 Bass kernel for nn_DataEmbedding_Stats (v2).

Computation: rolling-window stats (window=24, replicate-padded) over
x (B,S,7) -> 35 features -> circular conv1d(k=3) -> (B,S,512).

Strategy (8 NeuronCores, data parallel over batch, 4 batches/core):
 - 2 super-slabs (seq 0..2079 / 2080..4095), X loaded as bf16 via
   SWDGE cast-DMA into [112, E] tiles, partition = 28j + 7b + c.
 - rolling stats via log-doubling shifted tensor_tensor ladders,
   all in bf16 (2x DVE rate); sum/sq/max/min chains + var/std.
 - hop1: STK [112, 5*E] -> ST2A [28, 5*4104] (col = seq+2), host
   wrap stats for seq 4094/4095, high-wrap for seq 0/1.
 - hop2: F3 [106, 2048] per (batch, group): 3 tap-gathers from ST2A
   (4KB runs) + ones row (bias); group g covers pos 2048g..2048g+2047.
 - matmul weight-stationary: lhsT = wt[:,128dc:+128] (bias row 105),
   rhs = F3 slice [106, 512]; PSUM [128,1024] f32 pairs; drains
   (f32->bf16 cast) split 5:3 across ACT/DVE; output DMA per
   (b,g,dc) [128,2048] bf16 into flat y [128, 65536] (4KB runs);
   host re-assembles (b, s, d) and upcasts.
"""

import numpy as np

try:
    import concourse.bass as bass  # noqa: F401
except ImportError:
    import sys

    for _p in ("/opt/trn_rl_repo", "/root/.axon_site/_ro/trn_rl_repo"):
        if _p not in sys.path:
            sys.path.insert(0, _p)

B, S, C, W, D = 32, 4096, 7, 24, 512
NCORES = 8
BSH = B // NCORES          # batches per core
NF = 5 * C                 # 35 features
K = 3 * NF + 1             # 106 contraction rows (ones/bias row last)
HALO = W - 1               # 23
STW = 4104                 # per-stat block width in ST2A (col = seq + 2)
F3W = 2048                 # F3 cols = positions per group
NG = 2                     # output groups of 2048 positions
NDC = 4                    # d_model chunks of 128
# super-slabs: (seq base, chunk len, n chunks); E = chunk + HALO
SSPEC = ((0, 520, 4), (2080, 504, 4))

_CACHE = {}


def _build():
    import concourse.bacc as bacc
    import concourse.tile as tile
    from concourse import mybir

    f32 = mybir.dt.float32
    bf16 = mybir.dt.bfloat16
    Alu = mybir.AluOpType
    Act = mybir.ActivationFunctionType

    nc = bacc.Bacc(
        "TRN2",
        target_bir_lowering=False,
        debug=False,
        enable_asserts=False,
        num_devices=NCORES,
    )

    x_d = nc.dram_tensor("x", (BSH, C, S), f32, kind="ExternalInput")
    wt_d = nc.dram_tensor("wt", (K, D), bf16, kind="ExternalInput")
    ones_d = nc.dram_tensor("ones", (1, F3W), bf16, kind="ExternalInput")
    wrap_d = nc.dram_tensor("wrap", (28, 10), bf16, kind="ExternalInput")
    y_d = nc.dram_tensor(
        "y", (128, BSH * NG * NDC * F3W), bf16, kind="ExternalOutput"
    )

    with tile.TileContext(nc) as tc:
        with (
            tc.tile_pool(name="const", bufs=1) as pco,
            tc.tile_pool(name="xp", bufs=1) as pxx,
            tc.tile_pool(name="scr", bufs=1) as pscr,
            tc.tile_pool(name="lad", bufs=1) as plad,
            tc.tile_pool(name="stk", bufs=1) as pstk,
            tc.tile_pool(name="st2", bufs=1) as pst2,
            tc.tile_pool(name="f3p", bufs=8) as pf3,
            tc.tile_pool(name="psum", bufs=2, space="PSUM") as pps,
            tc.tile_pool(name="outp", bufs=4) as pout,
        ):
            wt = pco.tile([K, D], bf16, tag="wt")
            ST2A = pst2.tile([32, 5 * STW], bf16, tag="ST2A")
            XB = [None] * 2
            STK = [None] * 2
            F3 = [[None] * NG for _ in range(BSH)]
            STAGE = [[None] * NG for _ in range(BSH)]

            def load_x(ss):
                # SWDGE (gpsimd) cast-loads: f32 HBM -> bf16 SBUF
                base, ch, nj = SSPEC[ss]
                E = ch + HALO
                Xn = pxx.tile([112, E], bf16, tag=f"X{ss}", name=f"X{ss}")
                XB[ss] = Xn
                for j in range(nj):
                    s0 = base + ch * j
                    if ss == 0 and j == 0:
                        nc.gpsimd.dma_start(
                            Xn[0:28, HALO:E],
                            x_d.ap()[:, :, 0:ch].rearrange("b c q -> (b c) q"),
                        )
                    else:
                        nc.gpsimd.dma_start(
                            Xn[28 * j : 28 * j + 28, :],
                            x_d.ap()[
                                :, :, s0 - HALO : s0 + ch
                            ].rearrange("b c q -> (b c) q"),
                        )

            def halo0():
                Xn = XB[0]
                # replicate x[b,0,c] into halo cols 0..22 (scalar2 AP must
                # be f32, so stage the column through a tiny f32 tile)
                hc = pscr.tile([28, 1], f32, tag="haloc")
                nc.vector.tensor_copy(hc[0:28, 0:1], Xn[0:28, HALO : HALO + 1])
                nc.vector.tensor_scalar(
                    Xn[0:28, 0:HALO],
                    Xn[0:28, HALO : 2 * HALO],
                    0.0,
                    hc[0:28, 0:1],
                    Alu.mult,
                    Alu.add,
                )

            shifts = (1, 3, 7, 15)
            deltas = (1, 2, 4, 8)

            def mk_lad(ss, chains):
                _, ch, _ = SSPEC[ss]
                E = ch + HALO
                return {
                    c: [
                        plad.tile([112, E], bf16, tag=f"{c}{l}_{ss}",
                                  name=f"{c}{l}_{ss}")
                        for l in range(4)
                    ]
                    for c in chains
                }

            def tt(dst, d0, a, a0, bs, b0, op, E):
                nc.vector.tensor_tensor(
                    dst[0:112, d0:E], a[0:112, a0 : a0 + E - d0],
                    bs[0:112, b0 : b0 + E - d0], op,
                )

            def chain_levels(lad, src0, op, ss):
                _, ch, _ = SSPEC[ss]
                E = ch + HALO
                for l in range(4):
                    s, dl = shifts[l], deltas[l]
                    src = src0 if l == 0 else lad[l - 1]
                    tt(lad[l], s, src, s, src, s - dl, op, E)

            def stats_sumsq(ss):
                """sum + sq via f32-state prefix scans + shifted subtracts."""
                _, ch, _ = SSPEC[ss]
                E = ch + HALO
                Xn = XB[ss]
                SQX = pscr.tile([112, E], bf16, tag=f"SQX{ss}", name=f"SQX{ss}")
                nc.scalar.square(SQX[0:112, 0:E], Xn[0:112, 0:E])
                Pt = pscr.tile([112, E + 1], f32, tag=f"Pt{ss}", name=f"Pt{ss}")
                Qt = pscr.tile([112, E + 1], f32, tag=f"Qt{ss}", name=f"Qt{ss}")
                nc.vector.memset(Pt[0:112, 0:1], 0.0)
                nc.vector.memset(Qt[0:112, 0:1], 0.0)
                nc.vector.tensor_tensor_scan(
                    Pt[0:112, 1 : E + 1], Xn[0:112, 0:E], Xn[0:112, 0:E],
                    0.0, Alu.add, Alu.bypass,
                )
                nc.vector.tensor_tensor_scan(
                    Qt[0:112, 1 : E + 1], SQX[0:112, 0:E], SQX[0:112, 0:E],
                    0.0, Alu.add, Alu.bypass,
                )
                S24 = pscr.tile([112, E], bf16, tag=f"S24_{ss}", name=f"S24_{ss}")
                SQ24 = pscr.tile([112, E], bf16, tag=f"SQ24_{ss}",
                                 name=f"SQ24_{ss}")
                # S24[t] = P[t] - P[t-24]  (Pt is the prefix with a 0 col)
                nc.vector.tensor_tensor(
                    S24[0:112, HALO:E], Pt[0:112, HALO + 1 : E + 1],
                    Pt[0:112, 0 : E - HALO], Alu.subtract,
                )
                nc.vector.tensor_tensor(
                    SQ24[0:112, HALO:E], Qt[0:112, HALO + 1 : E + 1],
                    Qt[0:112, 0 : E - HALO], Alu.subtract,
                )
                return S24, SQ24

            def stats_rest(ss, S24, SQ24):
                """max/min chains, var/std path, stat blocks -> STK."""
                _, ch, _ = SSPEC[ss]
                E = ch + HALO
                Xn = XB[ss]
                stk = pstk.tile([112, 5 * E], bf16, tag=f"STK{ss}",
                                name=f"STK{ss}")
                STK[ss] = stk

                def blk(t):
                    return stk[0:112, E * t + HALO : E * (t + 1)]

                lad = mk_lad(ss, "MN")
                M_, N_ = lad["M"], lad["N"]
                for l in range(4):
                    s, dl = shifts[l], deltas[l]
                    msrc = Xn if l == 0 else M_[l - 1]
                    nsrc = Xn if l == 0 else N_[l - 1]
                    tt(M_[l], s, msrc, s, msrc, s - dl, Alu.max, E)
                    tt(N_[l], s, nsrc, s, nsrc, s - dl, Alu.min, E)
                nc.vector.tensor_tensor(
                    blk(2), M_[3][0:112, HALO:E], M_[2][0:112, 7 : E - 16],
                    Alu.max,
                )
                nc.vector.tensor_tensor(
                    blk(3), N_[3][0:112, HALO:E], N_[2][0:112, 7 : E - 16],
                    Alu.min,
                )
                # x and mean (raw S24; 1/24 folded into weights) blocks
                nc.vector.tensor_copy(blk(0), Xn[0:112, HALO:E])
                nc.vector.tensor_copy(blk(1), S24[0:112, HALO:E])
                # std = sqrt(max(SQ24 - S24^2/24, 0)/23)
                T4 = pscr.tile([112, E], bf16, tag=f"T4_{ss}", name=f"T4_{ss}")
                VV = pscr.tile([112, E], bf16, tag=f"VV_{ss}", name=f"VV_{ss}")
                VC = pscr.tile([112, E], bf16, tag=f"VC_{ss}", name=f"VC_{ss}")
                nc.scalar.activation(
                    T4[0:112, HALO:E], S24[0:112, HALO:E], Act.Square, 0.0,
                    float(W**-0.5),
                )
                tt(VV, HALO, SQ24, HALO, T4, HALO, Alu.subtract, E)
                nc.vector.tensor_scalar(
                    VC[0:112, HALO:E], VV[0:112, HALO:E], 0.0, None, Alu.max
                )
                nc.scalar.activation(
                    blk(4), VC[0:112, HALO:E], Act.Sqrt, 0.0, 1.0 / (W - 1)
                )

            def hop1(ss):
                base, ch, nj = SSPEC[ss]
                E = ch + HALO
                stk = STK[ss]
                for j in range(nj):
                    src = stk[28 * j : 28 * j + 28, :].rearrange(
                        "g (t m) -> g t m", m=E
                    )[:, :, HALO:E]
                    c0 = 2 + base + ch * j
                    dst = ST2A[0:28, :].rearrange("g (t m) -> g t m", m=STW)[
                        :, :, c0 : c0 + ch
                    ]
                    nc.gpsimd.dma_start(dst, src)

            def high_wrap():
                # seq 0,1 -> ST2A cols 4098..4099 (circular high wrap)
                nc.gpsimd.dma_start(
                    ST2A[0:28, :].rearrange("g (t m) -> g t m", m=STW)[
                        :, :, S + 2 : S + 4
                    ],
                    STK[0][0:28, :].rearrange(
                        "g (t m) -> g t m", m=SSPEC[0][1] + HALO
                    )[:, :, HALO : HALO + 2],
                )

            def wrap_low():
                # host stats for seq 4094/4095 -> ST2A cols 0..1
                nc.gpsimd.dma_start(
                    ST2A[0:28, :].rearrange("g (t m) -> g t m", m=STW)[:, :, 0:2],
                    wrap_d.ap().rearrange("g (t m) -> g t m", m=2),
                )

            def build_f3(b, g):
                f3 = F3[b][g]
                for k in range(3):
                    # F3 row 35k + 5c + t, col q <- ST2A col 2048g + k + 1 + q
                    c0 = F3W * g + k + 1
                    src = ST2A[7 * b : 7 * b + 7, :].rearrange(
                        "c (t m) -> c t m", m=STW
                    )[:, :, c0 : c0 + F3W]
                    nc.gpsimd.dma_start(f3[35 * k : 35 * k + 35, :], src)

            def mm_group(g):
                di = 0
                for b in range(BSH):
                    f3 = F3[b][g]
                    stage = pout.tile(
                        [128, NDC * F3W], bf16, tag="stage",
                        name=f"stage_{b}_{g}",
                    )
                    STAGE[b][g] = stage
                    for dc in range(NDC):
                        # 4-bank PSUM tile: 4 N=512 matmuls, one drain
                        ps = pps.tile([128, 2048], f32, tag="ps")
                        for ph in range(4):
                            nc.tensor.matmul(
                                ps[:, 512 * ph : 512 * (ph + 1)],
                                wt[0:K, 128 * dc : 128 * (dc + 1)],
                                f3[0:K, 512 * ph : 512 * ph + 512],
                                start=True,
                                stop=True,
                            )
                        col = dc * F3W
                        if di % 3 == 1:
                            nc.vector.tensor_copy(
                                stage[:, col : col + F3W], ps[:, 0:F3W]
                            )
                        else:
                            nc.scalar.copy(
                                stage[:, col : col + F3W], ps[:, 0:F3W]
                            )
                        di += 1
                        if dc % 2 == 1:
                            h = dc // 2
                            colbase = ((b * NG + g) * NDC + 2 * h) * F3W
                            nc.sync.dma_start(
                                y_d.ap()[:, colbase : colbase + 2 * F3W],
                                stage[:, 2 * h * F3W : (2 * h + 2) * F3W],
                            )

            # ---------------- pipeline
            load_x(0)
            load_x(1)
            nc.gpsimd.dma_start(wt[:], wt_d.ap())
            wrap_low()
            # pre-create F3 tiles; ones (bias) rows loaded early on gpsimd
            for b in range(BSH):
                for g in range(NG):
                    F3[b][g] = pf3.tile(
                        [K, F3W], bf16, tag="F3", name=f"f3_{b}_{g}"
                    )
                    nc.gpsimd.dma_start(F3[b][g][K - 1 : K, :], ones_d.ap())
            halo0()
            # SS0 full stats
            S24_0, SQ24_0 = stats_sumsq(0)
            stats_rest(0, S24_0, SQ24_0)
            hop1(0)
            high_wrap()
            for b in range(BSH):
                build_f3(b, 0)
            # SS1 sum/sq chains fill the DVE gap before G0 drains are ready
            S24_1, SQ24_1 = stats_sumsq(1)
            mm_group(0)
            stats_rest(1, S24_1, SQ24_1)
            hop1(1)
            for b in range(BSH):
                build_f3(b, 1)
            mm_group(1)

    nc.compile()
    return nc


def _prep_host(W_conv, b_conv):
    import ml_dtypes

    wt = np.empty((K, D), np.float32)
    wkf = np.ascontiguousarray(W_conv.transpose(2, 1, 0)).copy()  # (3, 35, 512)
    wkf[:, C : 2 * C, :] *= 1.0 / W  # fold mean = S24/24 into weights
    # row order within a tap: 5c + t (channel-major, matches F3 gather)
    wkf = wkf.reshape(3, 5, C, D).transpose(0, 2, 1, 3).reshape(3, NF, D)
    wt[: K - 1] = wkf.reshape(3 * NF, D)
    wt[K - 1] = b_conv.astype(np.float32)
    return wt.astype(ml_dtypes.bfloat16)


def _run(x, W_conv, b_conv, trace=False, **kw):
    from concourse import bass_utils

    if "nc" not in _CACHE:
        _CACHE["nc"] = _build()
    nc = _CACHE["nc"]

    wt = _prep_host(np.asarray(W_conv), np.asarray(b_conv))
    import ml_dtypes

    ones = np.ones((1, F3W), ml_dtypes.bfloat16)
    x = np.asarray(x, np.float32)
    # host stats for the circular-wrap cols (seq 4094/4095), [28, 10] per
    # core: row 7b+c, col 2t+e (t: x,sum,max,min,std; e: seq 4094+e)
    wraps = []
    for i in range(NCORES):
        wr = np.empty((BSH, C, 5, 2), np.float32)
        for b in range(BSH):
            for e in range(2):
                win = x[BSH * i + b, S - W - 1 + e : S - 1 + e, :]  # (24, 7)
                s24 = win.sum(0)
                var = np.maximum(
                    (win * win).sum(0) - s24 * s24 / W, 0.0
                ) / (W - 1)
                wr[b, :, 0, e] = x[BSH * i + b, S - 2 + e, :]
                wr[b, :, 1, e] = s24
                wr[b, :, 2, e] = win.max(0)
                wr[b, :, 3, e] = win.min(0)
                wr[b, :, 4, e] = np.sqrt(var)
        wraps.append(
            wr.reshape(BSH * C, 10).astype(ml_dtypes.bfloat16)
        )
    xt = np.ascontiguousarray(x.transpose(0, 2, 1))  # (B, C, S)
    in_maps = [
        {
            "x": xt[BSH * i : BSH * (i + 1)],
            "wt": wt,
            "ones": ones,
            "wrap": wraps[i],
        }
        for i in range(NCORES)
    ]
    res = bass_utils.run_bass_kernel_spmd(
        nc, in_maps, core_ids=list(range(NCORES)), trace=trace, **kw
    )
    outs = []
    for r in res.results:
        arr = np.asarray(r["y"], np.float32)  # (128, 65536)
        arr = arr.reshape(128, BSH, NG, NDC, F3W)
        # out[b, g*2048 + s, dc*128 + p]
        outs.append(
            np.ascontiguousarray(arr.transpose(1, 2, 4, 3, 0)).reshape(
                BSH, S, D
            )
        )
    out = np.concatenate(outs, axis=0)
    return out, res


def kernel(x, x_mark=None, W_conv=None, b_conv=None, **_unused):
    out, _ = _run(x, W_conv, b_conv, trace=False)
    return out


# revision 27
# speedup vs baseline: 1.4532x; 1.1524x over previous
"""Trainium2 Bass kernel for nn_DataEmbedding_Stats (v2).

Computation: rolling-window stats (window=24, replicate-padded) over
x (B,S,7) -> 35 features -> circular conv1d(k=3) -> (B,S,512).

Strategy (8 NeuronCores, data parallel over batch, 4 batches/core):
 - 2 super-slabs (seq 0..2079 / 2080..4095), X loaded as bf16 via
   SWDGE cast-DMA into [112, E] tiles, partition = 28j + 7b + c.
 - rolling stats via log-doubling shifted tensor_tensor ladders,
   all in bf16 (2x DVE rate); sum/sq/max/min chains + var/std.
 - hop1: STK [112, 5*E] -> ST2A [28, 5*4104] (col = seq+2), host
   wrap stats for seq 4094/4095, high-wrap for seq 0/1.
 - hop2: F3 [106, 2048] per (batch, group): 3 tap-gathers from ST2A
   (4KB runs) + ones row (bias); group g covers pos 2048g..2048g+2047.
 - matmul weight-stationary: lhsT = wt[:,128dc:+128] (bias row 105),
   rhs = F3 slice [106, 512]; PSUM [128,1024] f32 pairs; drains
   (f32->bf16 cast) split 5:3 across ACT/DVE; output DMA per
   (b,g,dc) [128,2048] bf16 into flat y [128, 65536] (4KB runs);
   host re-assembles (b, s, d) and upcasts.
"""

import numpy as np

try:
    import concourse.bass as bass  # noqa: F401
except ImportError:
    import sys

    for _p in ("/opt/trn_rl_repo", "/root/.axon_site/_ro/trn_rl_repo"):
        if _p not in sys.path:
            sys.path.insert(0, _p)

B, S, C, W, D = 32, 4096, 7, 24, 512
NCORES = 8
BSH = B // NCORES          # batches per core
NF = 5 * C                 # 35 features
K = 3 * NF + 1             # 106 contraction rows (ones/bias row last)
HALO = W - 1               # 23
STW = 4104                 # per-stat block width in ST2A (col = seq + 2)
F3W = 2048                 # F3 cols = positions per group
NG = 2                     # output groups of 2048 positions
NDC = 4                    # d_model chunks of 128
# super-slabs: (seq base, chunk len, n chunks); E = chunk + HALO
SSPEC = ((0, 520, 4), (2080, 504, 4))

_CACHE = {}


def _build():
    import concourse.bacc as bacc
    import concourse.tile as tile
    from concourse import mybir

    f32 = mybir.dt.float32
    bf16 = mybir.dt.bfloat16
    Alu = mybir.AluOpType
    Act = mybir.ActivationFunctionType

    nc = bacc.Bacc(
        "TRN2",
        target_bir_lowering=False,
        debug=False,
        enable_asserts=False,
        num_devices=NCORES,
    )

    x_d = nc.dram_tensor("x", (BSH, C, S), f32, kind="ExternalInput")
    wt_d = nc.dram_tensor("wt", (K, D), bf16, kind="ExternalInput")
    ones_d = nc.dram_tensor("ones", (1, F3W), bf16, kind="ExternalInput")
    wrap_d = nc.dram_tensor("wrap", (2, 70, 2), bf16, kind="ExternalInput")
    y_d = nc.dram_tensor(
        "y", (128, BSH * NG * NDC * F3W), bf16, kind="ExternalOutput"
    )

    with tile.TileContext(nc) as tc:
        with (
            tc.tile_pool(name="const", bufs=1) as pco,
            tc.tile_pool(name="xp", bufs=1) as pxx,
            tc.tile_pool(name="scr", bufs=1) as pscr,
            tc.tile_pool(name="lad", bufs=1) as plad,
            tc.tile_pool(name="stk", bufs=1) as pstk,
            tc.tile_pool(name="st2", bufs=1) as pst2,
            tc.tile_pool(name="f3p", bufs=6) as pf3,
            tc.tile_pool(name="psum", bufs=2, space="PSUM") as pps,
            tc.tile_pool(name="outp", bufs=6) as pout,
        ):
            wt = pco.tile([K, D], bf16, tag="wt")
            # per batch-pair stat plane: partition = 35*(b%2) + 5c + t,
            # col = seq + 2 (wrap cols 0..1, high wrap 4098..4099)
            ST2P = [
                pst2.tile([70, STW], bf16, tag=f"ST2P{p}", name=f"ST2P{p}")
                for p in range(2)
            ]
            XB = [None] * 2
            STK = [None] * 2
            F3 = [[None] * NG for _ in range(BSH)]
            STAGE = [[None] * NG for _ in range(BSH)]

            def load_x(ss):
                # SWDGE (gpsimd) cast-loads: f32 HBM -> bf16 SBUF
                base, ch, nj = SSPEC[ss]
                E = ch + HALO
                Xn = pxx.tile([112, E], bf16, tag=f"X{ss}", name=f"X{ss}")
                XB[ss] = Xn
                for j in range(nj):
                    s0 = base + ch * j
                    if ss == 0 and j == 0:
                        nc.gpsimd.dma_start(
                            Xn[0:28, HALO:E],
                            x_d.ap()[:, :, 0:ch].rearrange("b c q -> (b c) q"),
                        )
                    else:
                        nc.gpsimd.dma_start(
                            Xn[28 * j : 28 * j + 28, :],
                            x_d.ap()[
                                :, :, s0 - HALO : s0 + ch
                            ].rearrange("b c q -> (b c) q"),
                        )

            def halo0():
                Xn = XB[0]
                # replicate x[b,0,c] into halo cols 0..22 (scalar2 AP must
                # be f32, so stage the column through a tiny f32 tile)
                hc = pscr.tile([28, 1], f32, tag="haloc")
                nc.vector.tensor_copy(hc[0:28, 0:1], Xn[0:28, HALO : HALO + 1])
                nc.vector.tensor_scalar(
                    Xn[0:28, 0:HALO],
                    Xn[0:28, HALO : 2 * HALO],
                    0.0,
                    hc[0:28, 0:1],
                    Alu.mult,
                    Alu.add,
                )

            shifts = (1, 3, 7, 15)
            deltas = (1, 2, 4, 8)

            def mk_lad(ss, chains):
                _, ch, _ = SSPEC[ss]
                E = ch + HALO
                return {
                    c: [
                        plad.tile([112, E], bf16, tag=f"{c}{l}_{ss}",
                                  name=f"{c}{l}_{ss}")
                        for l in range(4)
                    ]
                    for c in chains
                }

            def tt(dst, d0, a, a0, bs, b0, op, E):
                nc.vector.tensor_tensor(
                    dst[0:112, d0:E], a[0:112, a0 : a0 + E - d0],
                    bs[0:112, b0 : b0 + E - d0], op,
                )

            def chain_levels(lad, src0, op, ss):
                _, ch, _ = SSPEC[ss]
                E = ch + HALO
                for l in range(4):
                    s, dl = shifts[l], deltas[l]
                    src = src0 if l == 0 else lad[l - 1]
                    tt(lad[l], s, src, s, src, s - dl, op, E)

            def stats_sumsq(ss):
                """sum + sq via f32-state prefix scans + shifted subtracts."""
                _, ch, _ = SSPEC[ss]
                E = ch + HALO
                Xn = XB[ss]
                SQX = pscr.tile([112, E], bf16, tag=f"SQX{ss}", name=f"SQX{ss}")
                nc.scalar.square(SQX[0:112, 0:E], Xn[0:112, 0:E])
                Pt = pscr.tile([112, E + 1], f32, tag=f"Pt{ss}", name=f"Pt{ss}")
                Qt = pscr.tile([112, E + 1], f32, tag=f"Qt{ss}", name=f"Qt{ss}")
                nc.vector.memset(Pt[0:112, 0:1], 0.0)
                nc.vector.memset(Qt[0:112, 0:1], 0.0)
                nc.vector.tensor_tensor_scan(
                    Pt[0:112, 1 : E + 1], Xn[0:112, 0:E], Xn[0:112, 0:E],
                    0.0, Alu.add, Alu.bypass,
                )
                nc.vector.tensor_tensor_scan(
                    Qt[0:112, 1 : E + 1], SQX[0:112, 0:E], SQX[0:112, 0:E],
                    0.0, Alu.add, Alu.bypass,
                )
                S24 = pscr.tile([112, E], bf16, tag=f"S24_{ss}", name=f"S24_{ss}")
                SQ24 = pscr.tile([112, E], bf16, tag=f"SQ24_{ss}",
                                 name=f"SQ24_{ss}")
                # S24[t] = P[t] - P[t-24]  (Pt is the prefix with a 0 col)
                nc.vector.tensor_tensor(
                    S24[0:112, HALO:E], Pt[0:112, HALO + 1 : E + 1],
                    Pt[0:112, 0 : E - HALO], Alu.subtract,
                )
                nc.vector.tensor_tensor(
                    SQ24[0:112, HALO:E], Qt[0:112, HALO + 1 : E + 1],
                    Qt[0:112, 0 : E - HALO], Alu.subtract,
                )
                return S24, SQ24

            def stats_rest(ss, S24, SQ24):
                """max/min chains, var/std path, stat blocks -> STK."""
                _, ch, _ = SSPEC[ss]
                E = ch + HALO
                Xn = XB[ss]
                stk = pstk.tile([112, 5 * E], bf16, tag=f"STK{ss}",
                                name=f"STK{ss}")
                STK[ss] = stk

                def blk(t):
                    return stk[0:112, E * t + HALO : E * (t + 1)]

                lad = mk_lad(ss, "MN")
                M_, N_ = lad["M"], lad["N"]
                for l in range(4):
                    s, dl = shifts[l], deltas[l]
                    msrc = Xn if l == 0 else M_[l - 1]
                    nsrc = Xn if l == 0 else N_[l - 1]
                    tt(M_[l], s, msrc, s, msrc, s - dl, Alu.max, E)
                    tt(N_[l], s, nsrc, s, nsrc, s - dl, Alu.min, E)
                nc.vector.tensor_tensor(
                    blk(2), M_[3][0:112, HALO:E], M_[2][0:112, 7 : E - 16],
                    Alu.max,
                )
                nc.vector.tensor_tensor(
                    blk(3), N_[3][0:112, HALO:E], N_[2][0:112, 7 : E - 16],
                    Alu.min,
                )
                # x and mean (raw S24; 1/24 folded into weights) blocks
                nc.vector.tensor_copy(blk(0), Xn[0:112, HALO:E])
                nc.vector.tensor_copy(blk(1), S24[0:112, HALO:E])
                # std = sqrt(max(SQ24 - S24^2/24, 0)/23)
                T4 = pscr.tile([112, E], bf16, tag=f"T4_{ss}", name=f"T4_{ss}")
                VV = pscr.tile([112, E], bf16, tag=f"VV_{ss}", name=f"VV_{ss}")
                VC = pscr.tile([112, E], bf16, tag=f"VC_{ss}", name=f"VC_{ss}")
                nc.scalar.activation(
                    T4[0:112, HALO:E], S24[0:112, HALO:E], Act.Square, 0.0,
                    float(W**-0.5),
                )
                tt(VV, HALO, SQ24, HALO, T4, HALO, Alu.subtract, E)
                nc.vector.tensor_scalar(
                    VC[0:112, HALO:E], VV[0:112, HALO:E], 0.0, None, Alu.max
                )
                nc.scalar.activation(
                    blk(4), VC[0:112, HALO:E], Act.Sqrt, 0.0, 1.0 / (W - 1)
                )

            def hop1(ss, engines=(nc.gpsimd, nc.sync)):
                # src [14, 5, ch] iterates (b2, c, t, m); dst [70, ch]
                # row-major (b2, c, t) matches exactly -> plain 2-D dst
                base, ch, nj = SSPEC[ss]
                E = ch + HALO
                stk = STK[ss]
                for j in range(nj):
                    c0 = 2 + base + ch * j
                    for p in range(2):
                        src = stk[
                            28 * j + 14 * p : 28 * j + 14 * p + 14, :
                        ].rearrange("q (t m) -> q t m", m=E)[:, :, HALO:E]
                        engines[p].dma_start(
                            ST2P[p][0:70, c0 : c0 + ch], src
                        )

            def high_wrap(engines=(nc.gpsimd, nc.sync)):
                # seq 0,1 -> ST2P cols 4098..4099 (circular high wrap)
                E0 = SSPEC[0][1] + HALO
                for p in range(2):
                    engines[p].dma_start(
                        ST2P[p][0:70, S + 2 : S + 4],
                        STK[0][14 * p : 14 * p + 14, :].rearrange(
                            "q (t m) -> q t m", m=E0
                        )[:, :, HALO : HALO + 2],
                    )

            def wrap_low():
                # host stats for seq 4094/4095 -> ST2P cols 0..1
                for p in range(2):
                    nc.sync.dma_start(
                        ST2P[p][0:70, 0:2], wrap_d.ap()[p]
                    )

            def build_f3(b, g):
                f3 = F3[b][g]
                for k in range(3):
                    # F3 row 35k + 5c + t, col q <- ST2P col 2048g + k + 1 + q
                    c0 = F3W * g + k + 1
                    src = ST2P[b // 2][
                        35 * (b % 2) : 35 * (b % 2) + 35, c0 : c0 + F3W
                    ]
                    nc.gpsimd.dma_start(f3[35 * k : 35 * k + 35, :], src)

            def mm_group(g):
                di = 0
                for b in range(BSH):
                    f3 = F3[b][g]
                    stage = pout.tile(
                        [128, NDC * F3W], bf16, tag="stage",
                        name=f"stage_{b}_{g}",
                    )
                    STAGE[b][g] = stage
                    for dc in range(NDC):
                        # 4-bank PSUM tile: 4 N=512 matmuls, one drain
                        ps = pps.tile([128, 2048], f32, tag="ps")
                        for ph in range(4):
                            nc.tensor.matmul(
                                ps[:, 512 * ph : 512 * (ph + 1)],
                                wt[0:K, 128 * dc : 128 * (dc + 1)],
                                f3[0:K, 512 * ph : 512 * ph + 512],
                                start=True,
                                stop=True,
                            )
                        col = dc * F3W
                        if di % 3 == 1:
                            nc.vector.tensor_copy(
                                stage[:, col : col + F3W], ps[:, 0:F3W]
                            )
                        else:
                            nc.scalar.copy(
                                stage[:, col : col + F3W], ps[:, 0:F3W]
                            )
                        di += 1
                        if dc % 2 == 1:
                            h = dc // 2
                            colbase = ((b * NG + g) * NDC + 2 * h) * F3W
                            nc.sync.dma_start(
                                y_d.ap()[:, colbase : colbase + 2 * F3W],
                                stage[:, 2 * h * F3W : (2 * h + 2) * F3W],
                            )

            # ---------------- pipeline
            load_x(0)
            nc.gpsimd.dma_start(wt[:], wt_d.ap())
            load_x(1)
            wrap_low()
            # pre-create F3 tiles; ones (bias) rows loaded early on gpsimd
            for b in range(BSH):
                for g in range(NG):
                    F3[b][g] = pf3.tile(
                        [K, F3W], bf16, tag="F3", name=f"f3_{b}_{g}"
                    )
                    nc.sync.dma_start(F3[b][g][K - 1 : K, :], ones_d.ap())
            halo0()
            # SS0 full stats
            S24_0, SQ24_0 = stats_sumsq(0)
            stats_rest(0, S24_0, SQ24_0)
            hop1(0)
            high_wrap()
            for b in range(BSH):
                build_f3(b, 0)
            # SS1 sum/sq chains fill the DVE gap before G0 drains are ready
            S24_1, SQ24_1 = stats_sumsq(1)
            mm_group(0)
            stats_rest(1, S24_1, SQ24_1)
            hop1(1, engines=(nc.gpsimd, nc.gpsimd))
            for b in range(BSH):
                build_f3(b, 1)
            mm_group(1)

    nc.compile()
    return nc


def _prep_host(W_conv, b_conv):
    import ml_dtypes

    wt = np.empty((K, D), np.float32)
    wkf = np.ascontiguousarray(W_conv.transpose(2, 1, 0)).copy()  # (3, 35, 512)
    wkf[:, C : 2 * C, :] *= 1.0 / W  # fold mean = S24/24 into weights
    # row order within a tap: 5c + t (channel-major, matches F3 gather)
    wkf = wkf.reshape(3, 5, C, D).transpose(0, 2, 1, 3).reshape(3, NF, D)
    wt[: K - 1] = wkf.reshape(3 * NF, D)
    wt[K - 1] = b_conv.astype(np.float32)
    return wt.astype(ml_dtypes.bfloat16)


def _run(x, W_conv, b_conv, trace=False, **kw):
    from concourse import bass_utils

    if "nc" not in _CACHE:
        _CACHE["nc"] = _build()
    nc = _CACHE["nc"]

    wt = _prep_host(np.asarray(W_conv), np.asarray(b_conv))
    import ml_dtypes

    ones = np.ones((1, F3W), ml_dtypes.bfloat16)
    x = np.asarray(x, np.float32)
    # host stats for the circular-wrap cols (seq 4094/4095), [28, 10] per
    # core: row 7b+c, col 2t+e (t: x,sum,max,min,std; e: seq 4094+e)
    wraps = []
    for i in range(NCORES):
        wr = np.empty((BSH, C, 5, 2), np.float32)
        for b in range(BSH):
            for e in range(2):
                win = x[BSH * i + b, S - W - 1 + e : S - 1 + e, :]  # (24, 7)
                s24 = win.sum(0)
                var = np.maximum(
                    (win * win).sum(0) - s24 * s24 / W, 0.0
                ) / (W - 1)
                wr[b, :, 0, e] = x[BSH * i + b, S - 2 + e, :]
                wr[b, :, 1, e] = s24
                wr[b, :, 2, e] = win.max(0)
                wr[b, :, 3, e] = win.min(0)
                wr[b, :, 4, e] = np.sqrt(var)
        # wr is (BSH, C, 5, 2) = (b, c, t, e) -> [pair, 35*b2+5c+t, e]
        wraps.append(
            np.ascontiguousarray(
                wr.reshape(2, 2, C, 5, 2)
            ).reshape(2, 70, 2).astype(ml_dtypes.bfloat16)
        )
    xt = np.ascontiguousarray(x.transpose(0, 2, 1))  # (B, C, S)
    in_maps = [
        {
            "x": xt[BSH * i : BSH * (i + 1)],
            "wt": wt,
            "ones": ones,
            "wrap": wraps[i],
        }
        for i in range(NCORES)
    ]
    res = bass_utils.run_bass_kernel_spmd(
        nc, in_maps, core_ids=list(range(NCORES)), trace=trace, **kw
    )
    outs = []
    for r in res.results:
        arr = np.asarray(r["y"], np.float32)  # (128, 65536)
        arr = arr.reshape(128, BSH, NG, NDC, F3W)
        # out[b, g*2048 + s, dc*128 + p]
        outs.append(
            np.ascontiguousarray(arr.transpose(1, 2, 4, 3, 0)).reshape(
                BSH, S, D
            )
        )
    out = np.concatenate(outs, axis=0)
    return out, res


def kernel(x, x_mark=None, W_conv=None, b_conv=None, **_unused):
    out, _ = _run(x, W_conv, b_conv, trace=False)
    return out
